# revision 8
# baseline (speedup 1.0000x reference)
"""GNN message-passing kernel for TRN2, 8-core SPMD (self-contained).

Design (v2):
- Node rows sharded 8 ways (NS=N/8 per core), edge rows too (ES=E/8).
- All gathers via gpsimd.dma_gather (int16 wrapped indices precomputed on the
  host). Nodes/edges are degree-sorted per shard (2-level key) so that whole
  4-slot gather chunks beyond each 128-row block's max degree are skipped --
  cutting gather descriptors ~30%. Index padding inside kept chunks points at
  a dedicated zero row (node tables get row N=8192); the edge table has no
  spare int16 index, so eid padding keeps the row-0 rank-1 correction.
- Per 128-row block:
    * dma_gather neighbor node rows / incident edge rows -> SBUF [128, nj*D]
    * masked mean via DVE pairwise folds (+ eid row-0 correction)
    * PE transposes build X^T; linear layer on PE; ReLU fused in the
      psum->SBUF copy on the scalar engine
    * store node-major result to shard staging; AllGather to next table
      (the large edge-table AllGather is split 4-ways so it pipelines with
      the producing edge blocks).
"""
import sys

sys.path.insert(0, '/opt/trn_rl_repo')

import numpy as np
import concourse.bass as bass
import concourse.mybir as mybir
from concourse import tile
from concourse.bacc import Bacc
from concourse.masks import make_identity

F32 = mybir.dt.float32
I16 = mybir.dt.int16
P = 128
QJ = 4  # j-columns per gather call


class Cfg:
    def __init__(self, N=8192, E=32768, D=512, DEG=16, DEP=8, K=3, CORES=8):
        self.N, self.E, self.D = N, E, D
        self.DEG, self.DEP, self.K, self.CORES = DEG, DEP, K, CORES
        self.NS = N // CORES
        self.ES = E // CORES
        self.NT = (self.NS + 1) * CORES  # node table rows (incl. zero rows)
        self.NB = self.NS // P
        self.EB = self.ES // P
        self.DC = D // P          # feature chunks per D
        self.KCN = (2 * D) // P   # contraction chunks, node linear
        self.KCE = (3 * D) // P   # contraction chunks, edge linear
        assert self.NS % P == 0 and self.ES % P == 0 and D % P == 0
        assert self.NT <= 32767 and E <= 32768  # int16 dma_gather indices


def build(cfg: Cfg, meta, gbufs=9, table_bf16=True, eag=4):
    """meta: per-block kept-chunk counts (host-derived from the actual
    degree distribution): kept_adj_fw/bw, kept_eid_fw/bw [NB] and
    kept_f/kept_b [EB], plus idx tensor widths."""
    N, E, D = cfg.N, cfg.E, cfg.D
    DEG, DEP, K, CORES = cfg.DEG, cfg.DEP, cfg.K, cfg.CORES
    NS, ES, NB, EB = cfg.NS, cfg.ES, cfg.NB, cfg.EB
    NT = cfg.NT
    DC, KCN, KCE = cfg.DC, cfg.KCN, cfg.KCE
    TDT = mybir.dt.bfloat16 if table_bf16 else F32

    ka_fw, ke_fw = meta["kept_adj_fw"], meta["kept_eid_fw"]
    ka_bw, ke_bw = meta["kept_adj_bw"], meta["kept_eid_bw"]
    kf_e, kb_e = meta["kept_f"], meta["kept_b"]

    def offs(kept):
        o, out = 0, []
        for k in kept:
            out.append(o)
            o += k * QJ * 8
        return out, o

    oa_fw, wa_fw = offs(ka_fw)
    oe_fw, we_fw = offs(ke_fw)
    oa_bw, wa_bw = offs(ka_bw)
    oe_bw, we_bw = offs(ke_bw)
    of_e, wf_e = offs(kf_e)
    ob_e, wb_e = offs(kb_e)

    nc = Bacc("TRN2", target_bir_lowering=False, debug=False,
              num_devices=CORES, num_swdge_queues=4,
              dynamic_dma_scratch_size=40960)

    # ---- external inputs ----
    fw_tab0 = nc.dram_tensor("fw_tab0", [NT, D], TDT, kind="ExternalInput")
    bw_tab0 = nc.dram_tensor("bw_tab0", [NT, D], TDT, kind="ExternalInput")
    e_tab0 = nc.dram_tensor("e_tab0", [E, D], TDT, kind="ExternalInput")
    fw_own0 = nc.dram_tensor("fw_own0", [NS, D], F32, kind="ExternalInput")
    bw_own0 = nc.dram_tensor("bw_own0", [NS, D], F32, kind="ExternalInput")
    e_own0 = nc.dram_tensor("e_own0", [ES, D], F32, kind="ExternalInput")
    fw_adj_g = nc.dram_tensor("fw_adj_g", [P, wa_fw], I16, kind="ExternalInput")
    bw_adj_g = nc.dram_tensor("bw_adj_g", [P, wa_bw], I16, kind="ExternalInput")
    fw_eid_g = nc.dram_tensor("fw_eid_g", [P, we_fw], I16, kind="ExternalInput")
    bw_eid_g = nc.dram_tensor("bw_eid_g", [P, we_bw], I16, kind="ExternalInput")
    fw_dep_g = nc.dram_tensor("fw_dep_g", [P, wf_e], I16, kind="ExternalInput")
    bw_dep_g = nc.dram_tensor("bw_dep_g", [P, wb_e], I16, kind="ExternalInput")
    rcn_fw = nc.dram_tensor("rcn_fw", [NS, 1], F32, kind="ExternalInput")
    rcn_bw = nc.dram_tensor("rcn_bw", [NS, 1], F32, kind="ExternalInput")
    rce_fw = nc.dram_tensor("rce_fw", [ES, 1], F32, kind="ExternalInput")
    rce_bw = nc.dram_tensor("rce_bw", [ES, 1], F32, kind="ExternalInput")
    ce_fw = nc.dram_tensor("ce_fw", [NS, 1], F32, kind="ExternalInput")
    ce_bw = nc.dram_tensor("ce_bw", [NS, 1], F32, kind="ExternalInput")
    Wfc = nc.dram_tensor("Wfc", [2 * D, D], F32, kind="ExternalInput")
    Wbc = nc.dram_tensor("Wbc", [2 * D, D], F32, kind="ExternalInput")
    Wedge = nc.dram_tensor("Wedge", [3 * D, D], F32, kind="ExternalInput")
    bfc = nc.dram_tensor("bfc", [1, D], F32, kind="ExternalInput")
    bbc = nc.dram_tensor("bbc", [1, D], F32, kind="ExternalInput")
    bedge = nc.dram_tensor("bedge", [1, D], F32, kind="ExternalInput")
    fw_out = nc.dram_tensor("fw_out", [NS, D], F32, kind="ExternalOutput")
    bw_out = nc.dram_tensor("bw_out", [NS, D], F32, kind="ExternalOutput")

    rg = [list(range(CORES))]

    with tile.TileContext(nc) as tc:
        with (
            tc.tile_pool(name="const", bufs=1) as cp,
            tc.tile_pool(name="gp", bufs=4) as gp,
            tc.tile_pool(name="ip", bufs=4) as ip,
            tc.tile_pool(name="xp", bufs=2) as xp,
            tc.tile_pool(name="sp", bufs=2) as sp,
            tc.tile_pool(name="fhp", bufs=3) as fhp,
            tc.tile_pool(name="r0p", bufs=2) as r0p,
            tc.tile_pool(name="bp", bufs=2) as bp,
            tc.tile_pool(name="pt", bufs=2, space="PSUM") as ptp,
            tc.tile_pool(name="po", bufs=2, space="PSUM") as pop,
            tc.tile_pool(name="dram", bufs=1, space="DRAM") as dp,
        ):
            # ---- constants ----
            ident = cp.tile([P, P], F32)
            make_identity(nc, ident[:])
            ones1 = cp.tile([1, P], F32)
            nc.gpsimd.memset(ones1[:], 1.0)
            if table_bf16:
                ones1t = cp.tile([1, P], TDT, name="ones1t")
                nc.gpsimd.memset(ones1t[:], 1.0)
            else:
                ones1t = ones1
            zrow = cp.tile([1, D], TDT, name="zrow")
            nc.gpsimd.memset(zrow[:], 0.0)

            def load_w(name, src, kc):
                t = cp.tile([P, kc * D], F32, name=name)
                for kk in range(kc):
                    nc.sync.dma_start(out=t[:, kk * D:(kk + 1) * D],
                                      in_=src[kk * P:(kk + 1) * P, :])
                return t

            wfc_t = load_w("wfc_t", Wfc, KCN)
            wbc_t = load_w("wbc_t", Wbc, KCN)
            we_t = load_w("we_t", Wedge, KCE)

            def load_flat(name, src, shape, dt):
                t = cp.tile(shape, dt, name=name)
                nc.sync.dma_start(out=t[:], in_=src[:])
                return t

            bfc_t = load_flat("bfc_t", bfc, [1, D], F32)
            bbc_t = load_flat("bbc_t", bbc, [1, D], F32)
            be_t = load_flat("be_t", bedge, [1, D], F32)
            idx_dram = {"fw_adj": fw_adj_g, "bw_adj": bw_adj_g,
                        "fw_eid": fw_eid_g, "bw_eid": bw_eid_g,
                        "fw_dep": fw_dep_g, "bw_dep": bw_dep_g}
            idx_sb = None
            if table_bf16:
                idx_sb = {nm: load_flat(f"ti_{nm}", t, [P, t.shape[1]], I16)
                          for nm, t in idx_dram.items()}

            def load_blocked(name, src, nb, w, dt):
                tt = cp.tile([P, nb * w], dt, name=name)
                for b in range(nb):
                    nc.sync.dma_start(out=tt[:, b * w:(b + 1) * w],
                                      in_=src[b * P:(b + 1) * P, :])
                return tt

            rc_t = {nm: load_blocked(f"t_{nm}", t, nb, 1, F32)
                    for nm, t, nb in (("rcn_fw", rcn_fw, NB), ("rcn_bw", rcn_bw, NB),
                                      ("rce_fw", rce_fw, EB), ("rce_bw", rce_bw, EB),
                                      ("ce_fw", ce_fw, NB), ("ce_bw", ce_bw, NB))}

            # ---- DRAM tables (internal) ----
            def mk_tab(name, rows):
                return dp.tile([rows, D], TDT, addr_space="Shared", name=name)

            fw_tabA = mk_tab("fw_tabA", NT)
            fw_tabB = mk_tab("fw_tabB", NT)
            bw_tabA = mk_tab("bw_tabA", NT)
            bw_tabB = mk_tab("bw_tabB", NT)
            e_tabA = mk_tab("e_tabA", E)
            e_tabB = mk_tab("e_tabB", E)
            fw_sh = dp.tile([NS, D], F32, name="fw_sh")
            bw_sh = dp.tile([NS, D], F32, name="bw_sh")
            e_sh = dp.tile([ES, D], F32, name="e_sh")
            # shard AG buffers carry a trailing zero row; the AllGather
            # interleaves them into the table as the per-core pad rows
            fw_shB = dp.tile([NS + 1, D], TDT, name="fw_shB")
            bw_shB = dp.tile([NS + 1, D], TDT, name="bw_shB")
            e_shB = dp.tile([ES, D], TDT, name="e_shB") if table_bf16 else e_sh
            nc.sync.dma_start(out=fw_shB[NS:NS + 1, :], in_=zrow[:])
            nc.sync.dma_start(out=bw_shB[NS:NS + 1, :], in_=zrow[:])

            fw_ntabs = [fw_tab0, fw_tabA, fw_tabB]
            bw_ntabs = [bw_tab0, bw_tabA, bw_tabB]
            e_tabs = [e_tab0, e_tabA, e_tabB]

            def bcast0(tab):
                """[128, D] tile with every partition = tab row 0."""
                r0 = r0p.tile([1, D], TDT, name="r0", tag="r0")
                nc.sync.dma_start(out=r0[:], in_=tab[0:1, :])
                ps = ptp.tile([P, D], F32, name="ps_t", tag="ps_t")
                nc.tensor.matmul(out=ps[:], lhsT=ones1t[:], rhs=r0[:],
                                 start=True, stop=True)
                b = bp.tile([P, D], F32, name="b0", tag="b0")
                nc.vector.tensor_copy(out=b[:], in_=ps[:])
                return b

            qctr = [0]

            def gather_q(tab, idx_nm, coloff, nj):
                if idx_sb is not None:
                    isl = idx_sb[idx_nm]
                    isl_ap = isl[:, coloff:coloff + nj * 8]
                else:
                    t = ip.tile([P, nj * 8], I16, name="isl", tag="isl")
                    nc.sync.dma_start(
                        out=t[:], in_=idx_dram[idx_nm][:, coloff:coloff + nj * 8])
                    isl_ap = t[:]
                g = gp.tile([P, nj * D], TDT, name="g", tag=f"g{nj}")
                qctr[0] = (qctr[0] + 1) % 4
                nc.gpsimd.dma_gather(
                    out_ap=g[:].rearrange("p (t e) -> p t e", e=D),
                    in_ap=tab[:],
                    idxs_ap=isl_ap,
                    num_idxs=nj * P,
                    num_idxs_reg=nj * P,
                    elem_size=D,
                    queue_num=qctr[0],
                    single_packet=(nj * P <= 512),
                )
                return g

            def gather_stream(tab, idx_nm, base, kept):
                """Merge 4-slot chunks into 8-slot gather calls."""
                gts = []
                h = 0
                while h < kept:
                    nj = 8 if kept - h >= 2 else 4
                    gts.append((gather_q(tab, idx_nm, base + h * QJ * 8, nj),
                                nj))
                    h += nj // QJ
                return gts

            ADD = mybir.AluOpType.add

            def mean_sbuf(gtiles, rc, b, corrs):
                """sm[128,D] = rc_b * (sum_j G_j + sum_i corr_i * B0_i)."""
                sm = sp.tile([P, D], F32, name="sm", tag="sm")
                init = False
                for (cx, B0x) in corrs:
                    if not init:
                        nc.vector.tensor_scalar_mul(sm[:], B0x[:], cx[:, b:b + 1])
                        init = True
                    else:
                        ct = sp.tile([P, D], F32, name="ct", tag="ct")
                        nc.vector.tensor_scalar_mul(ct[:], B0x[:], cx[:, b:b + 1])
                        nc.vector.tensor_tensor(out=sm[:], in0=sm[:], in1=ct[:], op=ADD)
                for g, slots in gtiles:
                    half = slots // 2
                    if table_bf16:
                        tq = sp.tile([P, half * D], F32, name=f"tq{slots}",
                                     tag=f"tq{slots}")
                        nc.vector.tensor_tensor(
                            out=tq[:], in0=g[:, 0:half * D],
                            in1=g[:, half * D:slots * D], op=ADD)
                        w = half
                        while w > 1:
                            hh = w // 2
                            nc.vector.tensor_tensor(
                                out=tq[:, 0:hh * D], in0=tq[:, 0:hh * D],
                                in1=tq[:, hh * D:w * D], op=ADD)
                            w = hh
                        src = tq[:, 0:D]
                    else:
                        w = slots
                        while w > 1:
                            hh = w // 2
                            nc.vector.tensor_tensor(
                                out=g[:, 0:hh * D], in0=g[:, 0:hh * D],
                                in1=g[:, hh * D:w * D], op=ADD)
                            w = hh
                        src = g[:, 0:D]
                    if not init:
                        nc.vector.tensor_copy(out=sm[:], in_=src)
                        init = True
                    else:
                        nc.vector.tensor_tensor(
                            out=sm[:], in0=sm[:], in1=src, op=ADD)
                assert init
                nc.vector.tensor_scalar_mul(sm[:], sm[:], rc[:, b:b + 1])
                return sm

            def transpose_into(xT, cbase, src_sb):
                pt = ptp.tile([P, DC * P], F32, name="ps_t")
                for c in range(DC):
                    nc.tensor.transpose(
                        out=pt[:, c * P:(c + 1) * P], in_=src_sb[:, c * P:(c + 1) * P],
                        identity=ident[:],
                    )
                nc.vector.tensor_copy(
                    out=xT[:, cbase * P:(cbase + DC) * P], in_=pt[:],
                )

            def linear(xT, kc, w_t, b_row, relu, out_sb):
                ps = pop.tile([P, D], F32, name="ps_o")
                for kk in range(kc):
                    nc.tensor.matmul(
                        out=ps[:], lhsT=xT[:, kk * P:(kk + 1) * P],
                        rhs=w_t[:, kk * D:(kk + 1) * D],
                        start=(kk == 0), stop=False,
                    )
                nc.tensor.matmul(
                    out=ps[:], lhsT=ones1[:], rhs=b_row[:], start=False, stop=True,
                )
                if relu:
                    nc.vector.tensor_scalar_max(out_sb[:], ps[:], 0.0)
                else:
                    nc.vector.tensor_copy(out=out_sb[:], in_=ps[:])

            def node_block(k, b, ntab, etab, own_src, a_nm, e_nm, rc_nm,
                           ce_nm, aoffs, akept, eoffs, ekept,
                           w_t, b_row, dst, dstB, eB0):
                relu = (k < K - 1)
                gts = gather_stream(ntab, a_nm, aoffs[b], akept[b])
                gts += gather_stream(etab, e_nm, eoffs[b], ekept[b])
                nf = sp.tile([P, D], F32, name="nf", tag="nf")
                nc.sync.dma_start(out=nf[:], in_=own_src[b * P:(b + 1) * P, :])

                sm = mean_sbuf(gts, rc_t[rc_nm], b, [(rc_t[ce_nm], eB0)])

                xT = xp.tile([P, KCN * P], F32, name="xT", tag="xT")
                transpose_into(xT, 0, nf)
                transpose_into(xT, DC, sm)

                fh = fhp.tile([P, D], F32, name="fh", tag="fh")
                linear(xT, KCN, w_t, b_row, relu, fh)
                nc.sync.dma_start(out=dst[b * P:(b + 1) * P, :], in_=fh[:])
                if dstB is not None:
                    fhb = fhp.tile([P, D], TDT, name="fhb", tag="fhb")
                    nc.vector.tensor_copy(out=fhb[:], in_=fh[:])
                    nc.sync.dma_start(out=dstB[b * P:(b + 1) * P, :], in_=fhb[:])

            def edge_block(u, b, fw_nt, bw_nt, own_src):
                gf = gather_stream(fw_nt, "fw_dep", of_e[b], kf_e[b])
                gb = gather_stream(bw_nt, "bw_dep", ob_e[b], kb_e[b])
                eo = sp.tile([P, D], F32, name="eo", tag="nf")
                nc.sync.dma_start(out=eo[:], in_=own_src[b * P:(b + 1) * P, :])

                smf = mean_sbuf(gf, rc_t["rce_fw"], b, [])
                smb = mean_sbuf(gb, rc_t["rce_bw"], b, [])

                xT = xp.tile([P, KCE * P], F32, name="xTe", tag="xT")
                transpose_into(xT, 0, eo)
                transpose_into(xT, DC, smf)
                transpose_into(xT, 2 * DC, smb)

                es = fhp.tile([P, D], F32, name="es", tag="fh")
                linear(xT, KCE, we_t, be_t, True, es)
                nc.sync.dma_start(out=e_sh[b * P:(b + 1) * P, :], in_=es[:])
                if table_bf16:
                    esb = fhp.tile([P, D], TDT, name="esb", tag="fhb")
                    nc.vector.tensor_copy(out=esb[:], in_=es[:])
                    nc.sync.dma_start(out=e_shB[b * P:(b + 1) * P, :], in_=esb[:])


            def allgather(src, dst_ap):
                nc.gpsimd.collective_compute(
                    "AllGather", mybir.AluOpType.bypass, replica_groups=rg,
                    ins=[src], outs=[dst_ap],
                )

            def allgather_e(dsttab):
                allgather(e_shB[:], dsttab[:])

            for k in range(K):
                last = (k == K - 1)
                fw_dst = fw_sh if not last else fw_out
                bw_dst = bw_sh if not last else bw_out
                fw_ownsrc = fw_own0 if k == 0 else fw_sh
                bw_ownsrc = bw_own0 if k == 0 else bw_sh
                eB0 = bcast0(e_tabs[k])
                for b in range(NB):
                    node_block(k, b, fw_ntabs[k], e_tabs[k], fw_ownsrc,
                               "fw_adj", "fw_eid", "rcn_fw", "ce_fw",
                               oa_fw, ka_fw, oe_fw, ke_fw,
                               wfc_t, bfc_t, fw_dst,
                               fw_shB if not last else None, eB0)
                if not last:
                    allgather(fw_shB[:], fw_ntabs[k + 1][:])
                for b in range(NB):
                    node_block(k, b, bw_ntabs[k], e_tabs[k], bw_ownsrc,
                               "bw_adj", "bw_eid", "rcn_bw", "ce_bw",
                               oa_bw, ka_bw, oe_bw, ke_bw,
                               wbc_t, bbc_t, bw_dst,
                               bw_shB if not last else None, eB0)
                if not last:
                    allgather(bw_shB[:], bw_ntabs[k + 1][:])
                    e_ownsrc = e_own0 if k == 0 else e_sh
                    for b in range(EB):
                        edge_block(k, b, fw_ntabs[k + 1], bw_ntabs[k + 1],
                                   e_ownsrc)
                    allgather_e(e_tabs[k + 1])

    nc.compile()
    return nc


def _valid_first(idx):
    """Per-row: valid entries first (stable), -1s last."""
    key = (idx < 0).astype(np.int32)
    order = np.argsort(key, axis=1, kind="stable")
    return np.take_along_axis(idx, order, axis=1)


def _pack_stream(rows, kept, pad_val):
    """rows [R, W] valid-first idx; kept[b] 4-slot chunks per 128-block.
    Returns [128, sum(kept)*32] int16 wrapped layout (pad -> pad_val)."""
    nb = rows.shape[0] // P
    cols = []
    for b in range(nb):
        k4 = kept[b] * QJ
        sub = rows[b * P:(b + 1) * P, :k4].copy()
        sub[sub < 0] = pad_val
        lst = sub.T.reshape(-1)                 # slot-major: i = j*128 + p
        wrapped = lst.reshape(-1, 16).T         # [16, k4*8]
        cols.append(np.tile(wrapped, (8, 1)).astype(np.int16))
    return np.concatenate(cols, axis=1) if cols else \
        np.zeros((P, 0), np.int16)


def _kept_chunks(deg, nb, min_chunks=0):
    """Per 128-block: ceil(max_deg/QJ) chunks."""
    out = []
    for b in range(nb):
        w = int(deg[b * P:(b + 1) * P].max()) if deg.size else 0
        out.append(max(min_chunks, (w + QJ - 1) // QJ))
    return out


def prep_inputs(cfg: Cfg, inputs: dict, table_bf16=True):
    N, E, D, DEG, DEP, CORES = cfg.N, cfg.E, cfg.D, cfg.DEG, cfg.DEP, cfg.CORES
    NS, ES, NB, EB = cfg.NS, cfg.ES, cfg.NB, cfg.EB
    f32 = np.float32

    fw = np.ascontiguousarray(np.asarray(inputs["fw_input"], f32))
    bw = np.ascontiguousarray(np.asarray(inputs["bw_input"], f32))
    ee = np.ascontiguousarray(np.asarray(inputs["edge_embs"], f32))

    idxs = {k: np.asarray(inputs[k], np.int64) for k in
            ("fw_adj", "bw_adj", "fw_edgeid", "bw_edgeid",
             "fw_edgedep", "bw_edgedep")}
    deg = {k: (v >= 0).sum(1) for k, v in idxs.items()}

    # ---- global degree sort, blocks dealt round-robin to cores so every
    # core's block b sees the same degree profile (kept counts stay tight) --
    def dealt_order(primary, secondary, rows_per):
        p1 = primary
        key = (-(p1 > (p1.max() // 2)).astype(np.int64)) * 10**6 - secondary
        order = np.argsort(key, kind="stable")           # new global -> old
        blocks = order.reshape(-1, P)
        return np.concatenate(
            [blocks[c::CORES].reshape(-1) for c in range(CORES)])

    ord_fw = dealt_order(deg["fw_adj"], deg["fw_edgeid"], NS)
    ord_bw = dealt_order(deg["bw_adj"], deg["bw_edgeid"], NS)
    ord_e = dealt_order(deg["fw_edgedep"], deg["bw_edgedep"], ES)

    def gmap(order, rows_per, stride=None):
        # old id order[c*rows_per+i] -> new TABLE id c*stride + i
        st = rows_per if stride is None else stride
        n = order.size
        cores = n // rows_per
        tgt = np.concatenate(
            [c * st + np.arange(rows_per) for c in range(cores)])
        m = np.empty(n, np.int64)
        m[order] = tgt
        return m

    gm_fw = gmap(ord_fw, NS, NS + 1)
    gm_bw = gmap(ord_bw, NS, NS + 1)
    gm_e = gmap(ord_e, ES)

    perm_fw, perm_bw = ord_fw, ord_bw

    def permute_rows(a, order, rows_per=None):
        return a[order]

    # remap index values (keep -1), then permute rows
    def remap(v, m):
        return np.where(v >= 0, m[np.maximum(v, 0)], -1)

    adj_fw = permute_rows(remap(idxs["fw_adj"], gm_fw), ord_fw)
    adj_bw = permute_rows(remap(idxs["bw_adj"], gm_bw), ord_bw)
    eid_fw = permute_rows(remap(idxs["fw_edgeid"], gm_e), ord_fw)
    eid_bw = permute_rows(remap(idxs["bw_edgeid"], gm_e), ord_bw)
    dep_fw = permute_rows(remap(idxs["fw_edgedep"], gm_fw), ord_e)
    dep_bw = permute_rows(remap(idxs["bw_edgedep"], gm_bw), ord_e)

    fw_s = permute_rows(fw, ord_fw)
    bw_s = permute_rows(bw, ord_bw)
    ee_s = permute_rows(ee, ord_e)

    deg_adj_fw = permute_rows(deg["fw_adj"], ord_fw)
    deg_eid_fw = permute_rows(deg["fw_edgeid"], ord_fw)
    deg_adj_bw = permute_rows(deg["bw_adj"], ord_bw)
    deg_eid_bw = permute_rows(deg["bw_edgeid"], ord_bw)
    deg_f_e = permute_rows(deg["fw_edgedep"], ord_e)
    deg_b_e = permute_rows(deg["bw_edgedep"], ord_e)

    adj_fw = _valid_first(adj_fw)
    adj_bw = _valid_first(adj_bw)
    eid_fw = _valid_first(eid_fw)
    eid_bw = _valid_first(eid_bw)
    dep_fw = _valid_first(dep_fw)
    dep_bw = _valid_first(dep_bw)

    if table_bf16:
        import ml_dtypes
        tdt = ml_dtypes.bfloat16
    else:
        tdt = f32
    zrow = np.zeros((1, D), f32)

    def mk_ntab(a):
        # interleave a zero row after each core's shard
        parts = []
        for c in range(CORES):
            parts.append(a[c * NS:(c + 1) * NS])
            parts.append(zrow)
        return np.concatenate(parts, 0).astype(tdt)

    fw_tab = mk_ntab(fw_s)
    bw_tab = mk_ntab(bw_s)
    ee_tab = ee_s.astype(tdt)

    def rec(c):
        with np.errstate(divide="ignore"):
            return (1.0 / c.astype(f32)).astype(f32)

    rcn_fw_f = rec(deg_adj_fw + deg_eid_fw)
    rcn_bw_f = rec(deg_adj_bw + deg_eid_bw)
    rce_fw_f = rec(deg_f_e)
    rce_bw_f = rec(deg_b_e)

    meta = {}
    per_core = {c: {} for c in range(CORES)}
    kept_all = {}
    for nm, rows, degv, rows_per, nb in (
            ("kept_adj_fw", adj_fw, deg_adj_fw, NS, NB),
            ("kept_eid_fw", eid_fw, deg_eid_fw, NS, NB),
            ("kept_adj_bw", adj_bw, deg_adj_bw, NS, NB),
            ("kept_eid_bw", eid_bw, deg_eid_bw, NS, NB),
            ("kept_f", dep_fw, deg_f_e, ES, EB),
            ("kept_b", dep_bw, deg_b_e, ES, EB)):
        # kept chunks must be IDENTICAL across cores (single SPMD program):
        # take per-core max per block index
        kept_pc = [_kept_chunks(degv[c * rows_per:(c + 1) * rows_per], nb)
                   for c in range(CORES)]
        kept = [max(kept_pc[c][b] for c in range(CORES)) for b in range(nb)]
        meta[nm] = kept
        kept_all[nm] = kept

    # eid row-0 correction: counts row-0 pads inside kept chunks
    def ce_vals(deg_e, kept, rows_per):
        out = np.empty(rows_per * CORES, f32)
        for c in range(CORES):
            for b in range(len(kept)):
                sl = slice(c * rows_per + b * P, c * rows_per + (b + 1) * P)
                out[sl] = -(kept[b] * QJ - deg_e[sl])
        return out

    ce_fw_f = ce_vals(deg_eid_fw, kept_all["kept_eid_fw"], NS)
    ce_bw_f = ce_vals(deg_eid_bw, kept_all["kept_eid_bw"], NS)

    Wfc = np.ascontiguousarray(np.asarray(inputs["Wfc"], f32))
    Wbc = np.ascontiguousarray(np.asarray(inputs["Wbc"], f32))
    Wedge = np.ascontiguousarray(np.asarray(inputs["Wedge"], f32))
    bfc = np.asarray(inputs["bfc"], f32).reshape(1, D)
    bbc = np.asarray(inputs["bbc"], f32).reshape(1, D)
    bedge = np.asarray(inputs["bedge"], f32).reshape(1, D)

    in_maps = []
    for c in range(CORES):
        nsl = slice(c * NS, (c + 1) * NS)
        esl = slice(c * ES, (c + 1) * ES)
        in_maps.append({
            "fw_tab0": fw_tab, "bw_tab0": bw_tab, "e_tab0": ee_tab,
            "fw_own0": fw_s[nsl].copy(), "bw_own0": bw_s[nsl].copy(),
            "e_own0": ee_s[esl].copy(),
            "fw_adj_g": _pack_stream(adj_fw[nsl], meta["kept_adj_fw"], NS),
            "bw_adj_g": _pack_stream(adj_bw[nsl], meta["kept_adj_bw"], NS),
            "fw_eid_g": _pack_stream(eid_fw[nsl], meta["kept_eid_fw"], 0),
            "bw_eid_g": _pack_stream(eid_bw[nsl], meta["kept_eid_bw"], 0),
            "fw_dep_g": _pack_stream(dep_fw[esl], meta["kept_f"], NS),
            "bw_dep_g": _pack_stream(dep_bw[esl], meta["kept_b"], NS),
            "rcn_fw": rcn_fw_f[nsl, None].copy(), "rcn_bw": rcn_bw_f[nsl, None].copy(),
            "rce_fw": rce_fw_f[esl, None].copy(), "rce_bw": rce_bw_f[esl, None].copy(),
            "ce_fw": ce_fw_f[nsl, None].copy(), "ce_bw": ce_bw_f[nsl, None].copy(),
            "Wfc": Wfc, "Wbc": Wbc, "Wedge": Wedge,
            "bfc": bfc, "bbc": bbc, "bedge": bedge,
        })
    return in_maps, meta, (perm_fw, perm_bw)


def assemble_outputs(cfg: Cfg, results, perms):
    ord_fw, ord_bw = perms
    fw = np.concatenate([results[c]["fw_out"] for c in range(cfg.CORES)], axis=0)
    bw = np.concatenate([results[c]["bw_out"] for c in range(cfg.CORES)], axis=0)
    ofw = np.empty_like(fw)
    obw = np.empty_like(bw)
    ofw[ord_fw] = fw
    obw[ord_bw] = bw
    return ofw, obw


# ======================= self-contained runner =======================
import os as _os
import types as _types


def _install_axon_prof():
    """Provide antenv.axon_hooks + NTFF hook so trace=True works under axon."""
    name = "antenv.axon_hooks"
    if name in sys.modules:
        return True
    try:
        mod = _types.ModuleType(name)
        mod._hook = None
        mod.set_axon_ntff_profile_hook = lambda h: setattr(mod, "_hook", h)
        mod.get_axon_ntff_profile_hook = lambda: mod._hook
        sys.modules[name] = mod
        import antenv
        antenv.axon_hooks = mod
        from trn_agent_boot.trn_boot import _ntff_profile_via_ctypes
        mod.set_axon_ntff_profile_hook(
            _ntff_profile_via_ctypes('/opt/axon/libaxon_pjrt.so'))
        return True
    except Exception:
        sys.modules.pop(name, None)
        return False


_CACHE = {}
LAST_EXEC_NS = None
LAST_PROFILE = None


def kernel(**inputs):
    """Full-input GNN forward on 8 TRN2 NeuronCores. Returns (fw, bw)."""
    global LAST_EXEC_NS, LAST_PROFILE
    from concourse.bass_utils import run_bass_kernel_spmd

    cfg = Cfg()
    bf16 = _os.environ.get("GNN_F32", "0") != "1"
    in_maps, meta, perms = prep_inputs(cfg, inputs, table_bf16=bf16)
    key = ("nc", tuple(tuple(meta[k]) for k in sorted(meta)))
    if _CACHE.get("key") != key:
        _CACHE["nc"] = build(cfg, meta, table_bf16=bf16)
        _CACHE["key"] = key
    nc = _CACHE["nc"]

    profile = _os.environ.get("GNN_PROFILE", "0") == "1"
    if profile:
        profile = _install_axon_prof()
    res = run_bass_kernel_spmd(nc, in_maps, core_ids=list(range(cfg.CORES)),
                               trace=profile)
    LAST_EXEC_NS = res.exec_time_ns
    LAST_PROFILE = res.profile_json
    if res.instructions_and_trace is not None:
        try:
            print("trace:", res.instructions_and_trace[1])
        except Exception:
            pass
    return assemble_outputs(cfg, res.results, perms)


# revision 9
# speedup vs baseline: 1.0232x; 1.0232x over previous
"""GNN message-passing kernel for TRN2, 8-core SPMD (self-contained).

Design (v2):
- Node rows sharded 8 ways (NS=N/8 per core), edge rows too (ES=E/8).
- All gathers via gpsimd.dma_gather (int16 wrapped indices precomputed on the
  host). Nodes/edges are degree-sorted per shard (2-level key) so that whole
  4-slot gather chunks beyond each 128-row block's max degree are skipped --
  cutting gather descriptors ~30%. Index padding inside kept chunks points at
  a dedicated zero row (node tables get row N=8192); the edge table has no
  spare int16 index, so eid padding keeps the row-0 rank-1 correction.
- Per 128-row block:
    * dma_gather neighbor node rows / incident edge rows -> SBUF [128, nj*D]
    * masked mean via DVE pairwise folds (+ eid row-0 correction)
    * PE transposes build X^T; linear layer on PE; ReLU fused in the
      psum->SBUF copy on the scalar engine
    * store node-major result to shard staging; AllGather to next table
      (the large edge-table AllGather is split 4-ways so it pipelines with
      the producing edge blocks).
"""
import sys

sys.path.insert(0, '/opt/trn_rl_repo')

import numpy as np
import concourse.bass as bass
import concourse.mybir as mybir
from concourse import tile
from concourse.bacc import Bacc
from concourse.masks import make_identity

F32 = mybir.dt.float32
I16 = mybir.dt.int16
P = 128
QJ = 4  # j-columns per gather call


class Cfg:
    def __init__(self, N=8192, E=32768, D=512, DEG=16, DEP=8, K=3, CORES=8):
        self.N, self.E, self.D = N, E, D
        self.DEG, self.DEP, self.K, self.CORES = DEG, DEP, K, CORES
        self.NS = N // CORES
        self.ES = E // CORES
        self.NT = (self.NS + 1) * CORES  # node table rows (incl. zero rows)
        self.NB = self.NS // P
        self.EB = self.ES // P
        self.DC = D // P          # feature chunks per D
        self.KCN = (2 * D) // P   # contraction chunks, node linear
        self.KCE = (3 * D) // P   # contraction chunks, edge linear
        assert self.NS % P == 0 and self.ES % P == 0 and D % P == 0
        assert self.NT <= 32767 and E <= 32768  # int16 dma_gather indices


def build(cfg: Cfg, meta, gbufs=9, table_bf16=True, eag=4):
    """meta: per-block kept-chunk counts (host-derived from the actual
    degree distribution): kept_adj_fw/bw, kept_eid_fw/bw [NB] and
    kept_f/kept_b [EB], plus idx tensor widths."""
    N, E, D = cfg.N, cfg.E, cfg.D
    DEG, DEP, K, CORES = cfg.DEG, cfg.DEP, cfg.K, cfg.CORES
    NS, ES, NB, EB = cfg.NS, cfg.ES, cfg.NB, cfg.EB
    NT = cfg.NT
    DC, KCN, KCE = cfg.DC, cfg.KCN, cfg.KCE
    TDT = mybir.dt.bfloat16 if table_bf16 else F32

    ka_fw, ke_fw = meta["kept_adj_fw"], meta["kept_eid_fw"]
    ka_bw, ke_bw = meta["kept_adj_bw"], meta["kept_eid_bw"]
    kf_e, kb_e = meta["kept_f"], meta["kept_b"]

    def offs(kept):
        o, out = 0, []
        for k in kept:
            out.append(o)
            o += k * QJ * 8
        return out, o

    oa_fw, wa_fw = offs(ka_fw)
    oe_fw, we_fw = offs(ke_fw)
    oa_bw, wa_bw = offs(ka_bw)
    oe_bw, we_bw = offs(ke_bw)
    of_e, wf_e = offs(kf_e)
    ob_e, wb_e = offs(kb_e)

    nc = Bacc("TRN2", target_bir_lowering=False, debug=False,
              num_devices=CORES, num_swdge_queues=4)

    # ---- external inputs ----
    fw_tab0 = nc.dram_tensor("fw_tab0", [NT, D], TDT, kind="ExternalInput")
    bw_tab0 = nc.dram_tensor("bw_tab0", [NT, D], TDT, kind="ExternalInput")
    e_tab0 = nc.dram_tensor("e_tab0", [E, D], TDT, kind="ExternalInput")
    fw_own0 = nc.dram_tensor("fw_own0", [NS, D], F32, kind="ExternalInput")
    bw_own0 = nc.dram_tensor("bw_own0", [NS, D], F32, kind="ExternalInput")
    e_own0 = nc.dram_tensor("e_own0", [ES, D], F32, kind="ExternalInput")
    fw_adj_g = nc.dram_tensor("fw_adj_g", [P, wa_fw], I16, kind="ExternalInput")
    bw_adj_g = nc.dram_tensor("bw_adj_g", [P, wa_bw], I16, kind="ExternalInput")
    fw_eid_g = nc.dram_tensor("fw_eid_g", [P, we_fw], I16, kind="ExternalInput")
    bw_eid_g = nc.dram_tensor("bw_eid_g", [P, we_bw], I16, kind="ExternalInput")
    fw_dep_g = nc.dram_tensor("fw_dep_g", [P, wf_e], I16, kind="ExternalInput")
    bw_dep_g = nc.dram_tensor("bw_dep_g", [P, wb_e], I16, kind="ExternalInput")
    rcn_fw = nc.dram_tensor("rcn_fw", [NS, 1], F32, kind="ExternalInput")
    rcn_bw = nc.dram_tensor("rcn_bw", [NS, 1], F32, kind="ExternalInput")
    rce_fw = nc.dram_tensor("rce_fw", [ES, 1], F32, kind="ExternalInput")
    rce_bw = nc.dram_tensor("rce_bw", [ES, 1], F32, kind="ExternalInput")
    ce_fw = nc.dram_tensor("ce_fw", [NS, 1], F32, kind="ExternalInput")
    ce_bw = nc.dram_tensor("ce_bw", [NS, 1], F32, kind="ExternalInput")
    Wfc = nc.dram_tensor("Wfc", [2 * D, D], F32, kind="ExternalInput")
    Wbc = nc.dram_tensor("Wbc", [2 * D, D], F32, kind="ExternalInput")
    Wedge = nc.dram_tensor("Wedge", [3 * D, D], F32, kind="ExternalInput")
    bfc = nc.dram_tensor("bfc", [1, D], F32, kind="ExternalInput")
    bbc = nc.dram_tensor("bbc", [1, D], F32, kind="ExternalInput")
    bedge = nc.dram_tensor("bedge", [1, D], F32, kind="ExternalInput")
    fw_out = nc.dram_tensor("fw_out", [NS, D], F32, kind="ExternalOutput")
    bw_out = nc.dram_tensor("bw_out", [NS, D], F32, kind="ExternalOutput")

    rg = [list(range(CORES))]

    with tile.TileContext(nc) as tc:
        with (
            tc.tile_pool(name="const", bufs=1) as cp,
            tc.tile_pool(name="gp", bufs=gbufs) as gp,
            tc.tile_pool(name="ip", bufs=10) as ip,
            tc.tile_pool(name="xp", bufs=2) as xp,
            tc.tile_pool(name="sp", bufs=4) as sp,
            tc.tile_pool(name="fhp", bufs=3) as fhp,
            tc.tile_pool(name="r0p", bufs=2) as r0p,
            tc.tile_pool(name="bp", bufs=4) as bp,
            tc.tile_pool(name="pt", bufs=2, space="PSUM") as ptp,
            tc.tile_pool(name="po", bufs=2, space="PSUM") as pop,
            tc.tile_pool(name="dram", bufs=1, space="DRAM") as dp,
        ):
            # ---- constants ----
            ident = cp.tile([P, P], F32)
            make_identity(nc, ident[:])
            ones1 = cp.tile([1, P], F32)
            nc.gpsimd.memset(ones1[:], 1.0)
            if table_bf16:
                ones1t = cp.tile([1, P], TDT, name="ones1t")
                nc.gpsimd.memset(ones1t[:], 1.0)
            else:
                ones1t = ones1
            zrow = cp.tile([1, D], TDT, name="zrow")
            nc.gpsimd.memset(zrow[:], 0.0)

            def load_w(name, src, kc):
                t = cp.tile([P, kc * D], F32, name=name)
                for kk in range(kc):
                    nc.sync.dma_start(out=t[:, kk * D:(kk + 1) * D],
                                      in_=src[kk * P:(kk + 1) * P, :])
                return t

            wfc_t = load_w("wfc_t", Wfc, KCN)
            wbc_t = load_w("wbc_t", Wbc, KCN)
            we_t = load_w("we_t", Wedge, KCE)

            def load_flat(name, src, shape, dt):
                t = cp.tile(shape, dt, name=name)
                nc.sync.dma_start(out=t[:], in_=src[:])
                return t

            bfc_t = load_flat("bfc_t", bfc, [1, D], F32)
            bbc_t = load_flat("bbc_t", bbc, [1, D], F32)
            be_t = load_flat("be_t", bedge, [1, D], F32)
            idx_dram = {"fw_adj": fw_adj_g, "bw_adj": bw_adj_g,
                        "fw_eid": fw_eid_g, "bw_eid": bw_eid_g,
                        "fw_dep": fw_dep_g, "bw_dep": bw_dep_g}
            idx_sb = None
            if table_bf16:
                idx_sb = {nm: load_flat(f"ti_{nm}", t, [P, t.shape[1]], I16)
                          for nm, t in idx_dram.items()}

            def load_blocked(name, src, nb, w, dt):
                tt = cp.tile([P, nb * w], dt, name=name)
                for b in range(nb):
                    nc.sync.dma_start(out=tt[:, b * w:(b + 1) * w],
                                      in_=src[b * P:(b + 1) * P, :])
                return tt

            rc_t = {nm: load_blocked(f"t_{nm}", t, nb, 1, F32)
                    for nm, t, nb in (("rcn_fw", rcn_fw, NB), ("rcn_bw", rcn_bw, NB),
                                      ("rce_fw", rce_fw, EB), ("rce_bw", rce_bw, EB),
                                      ("ce_fw", ce_fw, NB), ("ce_bw", ce_bw, NB))}

            # ---- DRAM tables (internal) ----
            def mk_tab(name, rows):
                return dp.tile([rows, D], TDT, addr_space="Shared", name=name)

            fw_tabA = mk_tab("fw_tabA", NT)
            fw_tabB = mk_tab("fw_tabB", NT)
            bw_tabA = mk_tab("bw_tabA", NT)
            bw_tabB = mk_tab("bw_tabB", NT)
            e_tabA = mk_tab("e_tabA", E)
            e_tabB = mk_tab("e_tabB", E)
            fw_sh = dp.tile([NS, D], F32, name="fw_sh")
            bw_sh = dp.tile([NS, D], F32, name="bw_sh")
            e_sh = dp.tile([ES, D], F32, name="e_sh")
            # shard AG buffers carry a trailing zero row; the AllGather
            # interleaves them into the table as the per-core pad rows
            fw_shB = dp.tile([NS + 1, D], TDT, name="fw_shB")
            bw_shB = dp.tile([NS + 1, D], TDT, name="bw_shB")
            e_shB = dp.tile([ES, D], TDT, name="e_shB") if table_bf16 else e_sh
            nc.sync.dma_start(out=fw_shB[NS:NS + 1, :], in_=zrow[:])
            nc.sync.dma_start(out=bw_shB[NS:NS + 1, :], in_=zrow[:])

            fw_ntabs = [fw_tab0, fw_tabA, fw_tabB]
            bw_ntabs = [bw_tab0, bw_tabA, bw_tabB]
            e_tabs = [e_tab0, e_tabA, e_tabB]

            def bcast0(tab):
                """[128, D] tile with every partition = tab row 0."""
                r0 = r0p.tile([1, D], TDT, name="r0", tag="r0")
                nc.sync.dma_start(out=r0[:], in_=tab[0:1, :])
                ps = ptp.tile([P, D], F32, name="ps_t", tag="ps_t")
                nc.tensor.matmul(out=ps[:], lhsT=ones1t[:], rhs=r0[:],
                                 start=True, stop=True)
                b = bp.tile([P, D], F32, name="b0", tag="b0")
                nc.vector.tensor_copy(out=b[:], in_=ps[:])
                return b

            qctr = [0]

            def gather_q(tab, idx_nm, coloff, nj):
                if idx_sb is not None:
                    isl = idx_sb[idx_nm]
                    isl_ap = isl[:, coloff:coloff + nj * 8]
                else:
                    t = ip.tile([P, nj * 8], I16, name="isl", tag="isl")
                    nc.sync.dma_start(
                        out=t[:], in_=idx_dram[idx_nm][:, coloff:coloff + nj * 8])
                    isl_ap = t[:]
                g = gp.tile([P, nj * D], TDT, name="g", tag="g")
                qctr[0] = (qctr[0] + 1) % 4
                nc.gpsimd.dma_gather(
                    out_ap=g[:].rearrange("p (t e) -> p t e", e=D),
                    in_ap=tab[:],
                    idxs_ap=isl_ap,
                    num_idxs=nj * P,
                    num_idxs_reg=nj * P,
                    elem_size=D,
                    queue_num=qctr[0],
                )
                return g

            ADD = mybir.AluOpType.add

            def mean_sbuf(gtiles, rc, b, corrs):
                """sm[128,D] = rc_b * (sum_j G_j + sum_i corr_i * B0_i)."""
                sm = sp.tile([P, D], F32, name="sm", tag="sm")
                init = False
                for (cx, B0x) in corrs:
                    if not init:
                        nc.vector.tensor_scalar_mul(sm[:], B0x[:], cx[:, b:b + 1])
                        init = True
                    else:
                        ct = sp.tile([P, D], F32, name="ct", tag="ct")
                        nc.vector.tensor_scalar_mul(ct[:], B0x[:], cx[:, b:b + 1])
                        nc.vector.tensor_tensor(out=sm[:], in0=sm[:], in1=ct[:], op=ADD)
                for g in gtiles:
                    if table_bf16:
                        tq = sp.tile([P, 2 * D], F32, name="tq", tag="tq")
                        nc.vector.tensor_tensor(
                            out=tq[:], in0=g[:, 0:2 * D], in1=g[:, 2 * D:4 * D], op=ADD)
                        nc.vector.tensor_tensor(
                            out=tq[:, 0:D], in0=tq[:, 0:D], in1=tq[:, D:2 * D], op=ADD)
                        if not init:
                            nc.vector.tensor_copy(out=sm[:], in_=tq[:, 0:D])
                            init = True
                        else:
                            nc.vector.tensor_tensor(
                                out=sm[:], in0=sm[:], in1=tq[:, 0:D], op=ADD)
                    else:
                        nc.vector.tensor_tensor(
                            out=g[:, 0:2 * D], in0=g[:, 0:2 * D], in1=g[:, 2 * D:4 * D], op=ADD)
                        nc.vector.tensor_tensor(
                            out=g[:, 0:D], in0=g[:, 0:D], in1=g[:, D:2 * D], op=ADD)
                        if not init:
                            nc.vector.tensor_copy(out=sm[:], in_=g[:, 0:D])
                            init = True
                        else:
                            nc.vector.tensor_tensor(
                                out=sm[:], in0=sm[:], in1=g[:, 0:D], op=ADD)
                assert init
                nc.vector.tensor_scalar_mul(sm[:], sm[:], rc[:, b:b + 1])
                return sm

            def transpose_into(xT, cbase, src_sb):
                pt = ptp.tile([P, DC * P], F32, name="ps_t")
                for c in range(DC):
                    nc.tensor.transpose(
                        out=pt[:, c * P:(c + 1) * P], in_=src_sb[:, c * P:(c + 1) * P],
                        identity=ident[:],
                    )
                nc.vector.tensor_copy(
                    out=xT[:, cbase * P:(cbase + DC) * P], in_=pt[:],
                )

            def linear(xT, kc, w_t, b_row, relu, out_sb):
                ps = pop.tile([P, D], F32, name="ps_o")
                for kk in range(kc):
                    nc.tensor.matmul(
                        out=ps[:], lhsT=xT[:, kk * P:(kk + 1) * P],
                        rhs=w_t[:, kk * D:(kk + 1) * D],
                        start=(kk == 0), stop=False,
                    )
                nc.tensor.matmul(
                    out=ps[:], lhsT=ones1[:], rhs=b_row[:], start=False, stop=True,
                )
                if relu:
                    nc.vector.tensor_scalar_max(out_sb[:], ps[:], 0.0)
                else:
                    nc.vector.tensor_copy(out=out_sb[:], in_=ps[:])

            def node_block(k, b, ntab, etab, own_src, a_nm, e_nm, rc_nm,
                           ce_nm, aoffs, akept, eoffs, ekept,
                           w_t, b_row, dst, dstB, eB0):
                relu = (k < K - 1)
                gts = []
                for h in range(akept[b]):
                    gts.append(gather_q(ntab, a_nm, aoffs[b] + h * QJ * 8, QJ))
                for h in range(ekept[b]):
                    gts.append(gather_q(etab, e_nm, eoffs[b] + h * QJ * 8, QJ))
                nf = sp.tile([P, D], F32, name="nf", tag="nf")
                nc.sync.dma_start(out=nf[:], in_=own_src[b * P:(b + 1) * P, :])

                sm = mean_sbuf(gts, rc_t[rc_nm], b, [(rc_t[ce_nm], eB0)])

                xT = xp.tile([P, KCN * P], F32, name="xT", tag="xT")
                transpose_into(xT, 0, nf)
                transpose_into(xT, DC, sm)

                fh = fhp.tile([P, D], F32, name="fh", tag="fh")
                linear(xT, KCN, w_t, b_row, relu, fh)
                nc.sync.dma_start(out=dst[b * P:(b + 1) * P, :], in_=fh[:])
                if dstB is not None:
                    fhb = fhp.tile([P, D], TDT, name="fhb", tag="fhb")
                    nc.vector.tensor_copy(out=fhb[:], in_=fh[:])
                    nc.sync.dma_start(out=dstB[b * P:(b + 1) * P, :], in_=fhb[:])

            def edge_block(u, b, fw_nt, bw_nt, own_src):
                gf = [gather_q(fw_nt, "fw_dep", of_e[b] + h * QJ * 8, QJ)
                      for h in range(kf_e[b])]
                gb = [gather_q(bw_nt, "bw_dep", ob_e[b] + h * QJ * 8, QJ)
                      for h in range(kb_e[b])]
                eo = sp.tile([P, D], F32, name="eo", tag="nf")
                nc.sync.dma_start(out=eo[:], in_=own_src[b * P:(b + 1) * P, :])

                smf = mean_sbuf(gf, rc_t["rce_fw"], b, [])
                smb = mean_sbuf(gb, rc_t["rce_bw"], b, [])

                xT = xp.tile([P, KCE * P], F32, name="xTe", tag="xT")
                transpose_into(xT, 0, eo)
                transpose_into(xT, DC, smf)
                transpose_into(xT, 2 * DC, smb)

                es = fhp.tile([P, D], F32, name="es", tag="fh")
                linear(xT, KCE, we_t, be_t, True, es)
                nc.sync.dma_start(out=e_sh[b * P:(b + 1) * P, :], in_=es[:])
                if table_bf16:
                    esb = fhp.tile([P, D], TDT, name="esb", tag="fhb")
                    nc.vector.tensor_copy(out=esb[:], in_=es[:])
                    nc.sync.dma_start(out=e_shB[b * P:(b + 1) * P, :], in_=esb[:])


            def allgather(src, dst_ap):
                nc.gpsimd.collective_compute(
                    "AllGather", mybir.AluOpType.bypass, replica_groups=rg,
                    ins=[src], outs=[dst_ap],
                )

            def allgather_e(dsttab):
                allgather(e_shB[:], dsttab[:])

            for k in range(K):
                last = (k == K - 1)
                fw_dst = fw_sh if not last else fw_out
                bw_dst = bw_sh if not last else bw_out
                fw_ownsrc = fw_own0 if k == 0 else fw_sh
                bw_ownsrc = bw_own0 if k == 0 else bw_sh
                eB0 = bcast0(e_tabs[k])
                for b in range(NB):
                    node_block(k, b, fw_ntabs[k], e_tabs[k], fw_ownsrc,
                               "fw_adj", "fw_eid", "rcn_fw", "ce_fw",
                               oa_fw, ka_fw, oe_fw, ke_fw,
                               wfc_t, bfc_t, fw_dst,
                               fw_shB if not last else None, eB0)
                if not last:
                    allgather(fw_shB[:], fw_ntabs[k + 1][:])
                for b in range(NB):
                    node_block(k, b, bw_ntabs[k], e_tabs[k], bw_ownsrc,
                               "bw_adj", "bw_eid", "rcn_bw", "ce_bw",
                               oa_bw, ka_bw, oe_bw, ke_bw,
                               wbc_t, bbc_t, bw_dst,
                               bw_shB if not last else None, eB0)
                if not last:
                    allgather(bw_shB[:], bw_ntabs[k + 1][:])
                    e_ownsrc = e_own0 if k == 0 else e_sh
                    for b in range(EB):
                        edge_block(k, b, fw_ntabs[k + 1], bw_ntabs[k + 1],
                                   e_ownsrc)
                    allgather_e(e_tabs[k + 1])

    nc.compile()
    return nc


def _valid_first(idx):
    """Per-row: valid entries first (stable), -1s last."""
    key = (idx < 0).astype(np.int32)
    order = np.argsort(key, axis=1, kind="stable")
    return np.take_along_axis(idx, order, axis=1)


def _pack_stream(rows, kept, pad_val):
    """rows [R, W] valid-first idx; kept[b] 4-slot chunks per 128-block.
    Returns [128, sum(kept)*32] int16 wrapped layout (pad -> pad_val)."""
    nb = rows.shape[0] // P
    cols = []
    for b in range(nb):
        k4 = kept[b] * QJ
        sub = rows[b * P:(b + 1) * P, :k4].copy()
        sub[sub < 0] = pad_val
        lst = sub.T.reshape(-1)                 # slot-major: i = j*128 + p
        wrapped = lst.reshape(-1, 16).T         # [16, k4*8]
        cols.append(np.tile(wrapped, (8, 1)).astype(np.int16))
    return np.concatenate(cols, axis=1) if cols else \
        np.zeros((P, 0), np.int16)


def _kept_chunks(deg, nb, min_chunks=0):
    """Per 128-block: ceil(max_deg/QJ) chunks."""
    out = []
    for b in range(nb):
        w = int(deg[b * P:(b + 1) * P].max()) if deg.size else 0
        out.append(max(min_chunks, (w + QJ - 1) // QJ))
    return out


def prep_inputs(cfg: Cfg, inputs: dict, table_bf16=True):
    N, E, D, DEG, DEP, CORES = cfg.N, cfg.E, cfg.D, cfg.DEG, cfg.DEP, cfg.CORES
    NS, ES, NB, EB = cfg.NS, cfg.ES, cfg.NB, cfg.EB
    f32 = np.float32

    fw = np.ascontiguousarray(np.asarray(inputs["fw_input"], f32))
    bw = np.ascontiguousarray(np.asarray(inputs["bw_input"], f32))
    ee = np.ascontiguousarray(np.asarray(inputs["edge_embs"], f32))

    idxs = {k: np.asarray(inputs[k], np.int64) for k in
            ("fw_adj", "bw_adj", "fw_edgeid", "bw_edgeid",
             "fw_edgedep", "bw_edgedep")}
    deg = {k: (v >= 0).sum(1) for k, v in idxs.items()}

    # ---- global degree sort, blocks dealt round-robin to cores so every
    # core's block b sees the same degree profile (kept counts stay tight) --
    def dealt_order(primary, secondary, rows_per):
        p1 = primary
        key = (-(p1 > (p1.max() // 2)).astype(np.int64)) * 10**6 - secondary
        order = np.argsort(key, kind="stable")           # new global -> old
        blocks = order.reshape(-1, P)
        return np.concatenate(
            [blocks[c::CORES].reshape(-1) for c in range(CORES)])

    ord_fw = dealt_order(deg["fw_adj"], deg["fw_edgeid"], NS)
    ord_bw = dealt_order(deg["bw_adj"], deg["bw_edgeid"], NS)
    ord_e = dealt_order(deg["fw_edgedep"], deg["bw_edgedep"], ES)

    def gmap(order, rows_per, stride=None):
        # old id order[c*rows_per+i] -> new TABLE id c*stride + i
        st = rows_per if stride is None else stride
        n = order.size
        cores = n // rows_per
        tgt = np.concatenate(
            [c * st + np.arange(rows_per) for c in range(cores)])
        m = np.empty(n, np.int64)
        m[order] = tgt
        return m

    gm_fw = gmap(ord_fw, NS, NS + 1)
    gm_bw = gmap(ord_bw, NS, NS + 1)
    gm_e = gmap(ord_e, ES)

    perm_fw, perm_bw = ord_fw, ord_bw

    def permute_rows(a, order, rows_per=None):
        return a[order]

    # remap index values (keep -1), then permute rows
    def remap(v, m):
        return np.where(v >= 0, m[np.maximum(v, 0)], -1)

    adj_fw = permute_rows(remap(idxs["fw_adj"], gm_fw), ord_fw)
    adj_bw = permute_rows(remap(idxs["bw_adj"], gm_bw), ord_bw)
    eid_fw = permute_rows(remap(idxs["fw_edgeid"], gm_e), ord_fw)
    eid_bw = permute_rows(remap(idxs["bw_edgeid"], gm_e), ord_bw)
    dep_fw = permute_rows(remap(idxs["fw_edgedep"], gm_fw), ord_e)
    dep_bw = permute_rows(remap(idxs["bw_edgedep"], gm_bw), ord_e)

    fw_s = permute_rows(fw, ord_fw)
    bw_s = permute_rows(bw, ord_bw)
    ee_s = permute_rows(ee, ord_e)

    deg_adj_fw = permute_rows(deg["fw_adj"], ord_fw)
    deg_eid_fw = permute_rows(deg["fw_edgeid"], ord_fw)
    deg_adj_bw = permute_rows(deg["bw_adj"], ord_bw)
    deg_eid_bw = permute_rows(deg["bw_edgeid"], ord_bw)
    deg_f_e = permute_rows(deg["fw_edgedep"], ord_e)
    deg_b_e = permute_rows(deg["bw_edgedep"], ord_e)

    adj_fw = _valid_first(adj_fw)
    adj_bw = _valid_first(adj_bw)
    eid_fw = _valid_first(eid_fw)
    eid_bw = _valid_first(eid_bw)
    dep_fw = _valid_first(dep_fw)
    dep_bw = _valid_first(dep_bw)

    if table_bf16:
        import ml_dtypes
        tdt = ml_dtypes.bfloat16
    else:
        tdt = f32
    zrow = np.zeros((1, D), f32)

    def mk_ntab(a):
        # interleave a zero row after each core's shard
        parts = []
        for c in range(CORES):
            parts.append(a[c * NS:(c + 1) * NS])
            parts.append(zrow)
        return np.concatenate(parts, 0).astype(tdt)

    fw_tab = mk_ntab(fw_s)
    bw_tab = mk_ntab(bw_s)
    ee_tab = ee_s.astype(tdt)

    def rec(c):
        with np.errstate(divide="ignore"):
            return (1.0 / c.astype(f32)).astype(f32)

    rcn_fw_f = rec(deg_adj_fw + deg_eid_fw)
    rcn_bw_f = rec(deg_adj_bw + deg_eid_bw)
    rce_fw_f = rec(deg_f_e)
    rce_bw_f = rec(deg_b_e)

    meta = {}
    per_core = {c: {} for c in range(CORES)}
    kept_all = {}
    for nm, rows, degv, rows_per, nb in (
            ("kept_adj_fw", adj_fw, deg_adj_fw, NS, NB),
            ("kept_eid_fw", eid_fw, deg_eid_fw, NS, NB),
            ("kept_adj_bw", adj_bw, deg_adj_bw, NS, NB),
            ("kept_eid_bw", eid_bw, deg_eid_bw, NS, NB),
            ("kept_f", dep_fw, deg_f_e, ES, EB),
            ("kept_b", dep_bw, deg_b_e, ES, EB)):
        # kept chunks must be IDENTICAL across cores (single SPMD program):
        # take per-core max per block index
        kept_pc = [_kept_chunks(degv[c * rows_per:(c + 1) * rows_per], nb)
                   for c in range(CORES)]
        kept = [max(kept_pc[c][b] for c in range(CORES)) for b in range(nb)]
        meta[nm] = kept
        kept_all[nm] = kept

    # eid row-0 correction: counts row-0 pads inside kept chunks
    def ce_vals(deg_e, kept, rows_per):
        out = np.empty(rows_per * CORES, f32)
        for c in range(CORES):
            for b in range(len(kept)):
                sl = slice(c * rows_per + b * P, c * rows_per + (b + 1) * P)
                out[sl] = -(kept[b] * QJ - deg_e[sl])
        return out

    ce_fw_f = ce_vals(deg_eid_fw, kept_all["kept_eid_fw"], NS)
    ce_bw_f = ce_vals(deg_eid_bw, kept_all["kept_eid_bw"], NS)

    Wfc = np.ascontiguousarray(np.asarray(inputs["Wfc"], f32))
    Wbc = np.ascontiguousarray(np.asarray(inputs["Wbc"], f32))
    Wedge = np.ascontiguousarray(np.asarray(inputs["Wedge"], f32))
    bfc = np.asarray(inputs["bfc"], f32).reshape(1, D)
    bbc = np.asarray(inputs["bbc"], f32).reshape(1, D)
    bedge = np.asarray(inputs["bedge"], f32).reshape(1, D)

    in_maps = []
    for c in range(CORES):
        nsl = slice(c * NS, (c + 1) * NS)
        esl = slice(c * ES, (c + 1) * ES)
        in_maps.append({
            "fw_tab0": fw_tab, "bw_tab0": bw_tab, "e_tab0": ee_tab,
            "fw_own0": fw_s[nsl].copy(), "bw_own0": bw_s[nsl].copy(),
            "e_own0": ee_s[esl].copy(),
            "fw_adj_g": _pack_stream(adj_fw[nsl], meta["kept_adj_fw"], NS),
            "bw_adj_g": _pack_stream(adj_bw[nsl], meta["kept_adj_bw"], NS),
            "fw_eid_g": _pack_stream(eid_fw[nsl], meta["kept_eid_fw"], 0),
            "bw_eid_g": _pack_stream(eid_bw[nsl], meta["kept_eid_bw"], 0),
            "fw_dep_g": _pack_stream(dep_fw[esl], meta["kept_f"], NS),
            "bw_dep_g": _pack_stream(dep_bw[esl], meta["kept_b"], NS),
            "rcn_fw": rcn_fw_f[nsl, None].copy(), "rcn_bw": rcn_bw_f[nsl, None].copy(),
            "rce_fw": rce_fw_f[esl, None].copy(), "rce_bw": rce_bw_f[esl, None].copy(),
            "ce_fw": ce_fw_f[nsl, None].copy(), "ce_bw": ce_bw_f[nsl, None].copy(),
            "Wfc": Wfc, "Wbc": Wbc, "Wedge": Wedge,
            "bfc": bfc, "bbc": bbc, "bedge": bedge,
        })
    return in_maps, meta, (perm_fw, perm_bw)


def assemble_outputs(cfg: Cfg, results, perms):
    ord_fw, ord_bw = perms
    fw = np.concatenate([results[c]["fw_out"] for c in range(cfg.CORES)], axis=0)
    bw = np.concatenate([results[c]["bw_out"] for c in range(cfg.CORES)], axis=0)
    ofw = np.empty_like(fw)
    obw = np.empty_like(bw)
    ofw[ord_fw] = fw
    obw[ord_bw] = bw
    return ofw, obw


# ======================= self-contained runner =======================
import os as _os
import types as _types


def _install_axon_prof():
    """Provide antenv.axon_hooks + NTFF hook so trace=True works under axon."""
    name = "antenv.axon_hooks"
    if name in sys.modules:
        return True
    try:
        mod = _types.ModuleType(name)
        mod._hook = None
        mod.set_axon_ntff_profile_hook = lambda h: setattr(mod, "_hook", h)
        mod.get_axon_ntff_profile_hook = lambda: mod._hook
        sys.modules[name] = mod
        import antenv
        antenv.axon_hooks = mod
        from trn_agent_boot.trn_boot import _ntff_profile_via_ctypes
        mod.set_axon_ntff_profile_hook(
            _ntff_profile_via_ctypes('/opt/axon/libaxon_pjrt.so'))
        return True
    except Exception:
        sys.modules.pop(name, None)
        return False


_CACHE = {}
LAST_EXEC_NS = None
LAST_PROFILE = None


def kernel(**inputs):
    """Full-input GNN forward on 8 TRN2 NeuronCores. Returns (fw, bw)."""
    global LAST_EXEC_NS, LAST_PROFILE
    from concourse.bass_utils import run_bass_kernel_spmd

    cfg = Cfg()
    bf16 = _os.environ.get("GNN_F32", "0") != "1"
    in_maps, meta, perms = prep_inputs(cfg, inputs, table_bf16=bf16)
    key = ("nc", tuple(tuple(meta[k]) for k in sorted(meta)))
    if _CACHE.get("key") != key:
        _CACHE["nc"] = build(cfg, meta, table_bf16=bf16)
        _CACHE["key"] = key
    nc = _CACHE["nc"]

    profile = _os.environ.get("GNN_PROFILE", "0") == "1"
    if profile:
        profile = _install_axon_prof()
    res = run_bass_kernel_spmd(nc, in_maps, core_ids=list(range(cfg.CORES)),
                               trace=profile)
    LAST_EXEC_NS = res.exec_time_ns
    LAST_PROFILE = res.profile_json
    if res.instructions_and_trace is not None:
        try:
            print("trace:", res.instructions_and_trace[1])
        except Exception:
            pass
    return assemble_outputs(cfg, res.results, perms)


# revision 10
# speedup vs baseline: 1.0656x; 1.0415x over previous
"""GNN message-passing kernel for TRN2, 8-core SPMD (self-contained).

Design (v2):
- Node rows sharded 8 ways (NS=N/8 per core), edge rows too (ES=E/8).
- All gathers via gpsimd.dma_gather (int16 wrapped indices precomputed on the
  host). Nodes/edges are degree-sorted per shard (2-level key) so that whole
  4-slot gather chunks beyond each 128-row block's max degree are skipped --
  cutting gather descriptors ~30%. Index padding inside kept chunks points at
  a dedicated zero row (node tables get row N=8192); the edge table has no
  spare int16 index, so eid padding keeps the row-0 rank-1 correction.
- Per 128-row block:
    * dma_gather neighbor node rows / incident edge rows -> SBUF [128, nj*D]
    * masked mean via DVE pairwise folds (+ eid row-0 correction)
    * PE transposes build X^T; linear layer on PE; ReLU fused in the
      psum->SBUF copy on the scalar engine
    * store node-major result to shard staging; AllGather to next table
      (the large edge-table AllGather is split 4-ways so it pipelines with
      the producing edge blocks).
"""
import sys

sys.path.insert(0, '/opt/trn_rl_repo')

import numpy as np
import concourse.bass as bass
import concourse.mybir as mybir
from concourse import tile
from concourse.bacc import Bacc
from concourse.masks import make_identity

F32 = mybir.dt.float32
I16 = mybir.dt.int16
P = 128
QJ = 4  # j-columns per gather call


class Cfg:
    def __init__(self, N=8192, E=32768, D=512, DEG=16, DEP=8, K=3, CORES=8):
        self.N, self.E, self.D = N, E, D
        self.DEG, self.DEP, self.K, self.CORES = DEG, DEP, K, CORES
        self.NS = N // CORES
        self.ES = E // CORES
        self.NT = (self.NS + 1) * CORES  # node table rows (incl. zero rows)
        self.NB = self.NS // P
        self.EB = self.ES // P
        self.DC = D // P          # feature chunks per D
        self.KCN = (2 * D) // P   # contraction chunks, node linear
        self.KCE = (3 * D) // P   # contraction chunks, edge linear
        assert self.NS % P == 0 and self.ES % P == 0 and D % P == 0
        assert self.NT <= 32767 and E <= 32768  # int16 dma_gather indices


def build(cfg: Cfg, meta, gbufs=9, table_bf16=True, eag=4):
    """meta: per-block kept-chunk counts (host-derived from the actual
    degree distribution): kept_adj_fw/bw, kept_eid_fw/bw [NB] and
    kept_f/kept_b [EB], plus idx tensor widths."""
    N, E, D = cfg.N, cfg.E, cfg.D
    DEG, DEP, K, CORES = cfg.DEG, cfg.DEP, cfg.K, cfg.CORES
    NS, ES, NB, EB = cfg.NS, cfg.ES, cfg.NB, cfg.EB
    NT = cfg.NT
    DC, KCN, KCE = cfg.DC, cfg.KCN, cfg.KCE
    TDT = mybir.dt.bfloat16 if table_bf16 else F32

    ka_fw, ke_fw = meta["kept_adj_fw"], meta["kept_eid_fw"]
    ka_bw, ke_bw = meta["kept_adj_bw"], meta["kept_eid_bw"]
    kf_e, kb_e = meta["kept_f"], meta["kept_b"]

    def offs(kept):
        # kept is per-block SLOT counts; 8 wrapped idx columns per slot
        o, out = 0, []
        for k in kept:
            out.append(o)
            o += k * 8
        return out, o

    oa_fw, wa_fw = offs(ka_fw)
    oe_fw, we_fw = offs(ke_fw)
    oa_bw, wa_bw = offs(ka_bw)
    oe_bw, we_bw = offs(ke_bw)
    of_e, wf_e = offs(kf_e)
    ob_e, wb_e = offs(kb_e)

    nc = Bacc("TRN2", target_bir_lowering=False, debug=False,
              num_devices=CORES, num_swdge_queues=4)

    # ---- external inputs ----
    fw_tab0 = nc.dram_tensor("fw_tab0", [NT, D], TDT, kind="ExternalInput")
    bw_tab0 = nc.dram_tensor("bw_tab0", [NT, D], TDT, kind="ExternalInput")
    e_tab0 = nc.dram_tensor("e_tab0", [E, D], TDT, kind="ExternalInput")
    fw_own0 = nc.dram_tensor("fw_own0", [NS, D], F32, kind="ExternalInput")
    bw_own0 = nc.dram_tensor("bw_own0", [NS, D], F32, kind="ExternalInput")
    e_own0 = nc.dram_tensor("e_own0", [ES, D], F32, kind="ExternalInput")
    fw_adj_g = nc.dram_tensor("fw_adj_g", [P, wa_fw], I16, kind="ExternalInput")
    bw_adj_g = nc.dram_tensor("bw_adj_g", [P, wa_bw], I16, kind="ExternalInput")
    fw_eid_g = nc.dram_tensor("fw_eid_g", [P, we_fw], I16, kind="ExternalInput")
    bw_eid_g = nc.dram_tensor("bw_eid_g", [P, we_bw], I16, kind="ExternalInput")
    fw_dep_g = nc.dram_tensor("fw_dep_g", [P, wf_e], I16, kind="ExternalInput")
    bw_dep_g = nc.dram_tensor("bw_dep_g", [P, wb_e], I16, kind="ExternalInput")
    rcn_fw = nc.dram_tensor("rcn_fw", [NS, 1], F32, kind="ExternalInput")
    rcn_bw = nc.dram_tensor("rcn_bw", [NS, 1], F32, kind="ExternalInput")
    rce_fw = nc.dram_tensor("rce_fw", [ES, 1], F32, kind="ExternalInput")
    rce_bw = nc.dram_tensor("rce_bw", [ES, 1], F32, kind="ExternalInput")
    ce_fw = nc.dram_tensor("ce_fw", [NS, 1], F32, kind="ExternalInput")
    ce_bw = nc.dram_tensor("ce_bw", [NS, 1], F32, kind="ExternalInput")
    Wfc = nc.dram_tensor("Wfc", [2 * D, D], F32, kind="ExternalInput")
    Wbc = nc.dram_tensor("Wbc", [2 * D, D], F32, kind="ExternalInput")
    Wedge = nc.dram_tensor("Wedge", [3 * D, D], F32, kind="ExternalInput")
    bfc = nc.dram_tensor("bfc", [1, D], F32, kind="ExternalInput")
    bbc = nc.dram_tensor("bbc", [1, D], F32, kind="ExternalInput")
    bedge = nc.dram_tensor("bedge", [1, D], F32, kind="ExternalInput")
    fw_out = nc.dram_tensor("fw_out", [NS, D], F32, kind="ExternalOutput")
    bw_out = nc.dram_tensor("bw_out", [NS, D], F32, kind="ExternalOutput")

    rg = [list(range(CORES))]

    with tile.TileContext(nc) as tc:
        with (
            tc.tile_pool(name="const", bufs=1) as cp,
            tc.tile_pool(name="gp", bufs=gbufs) as gp,
            tc.tile_pool(name="ip", bufs=10) as ip,
            tc.tile_pool(name="xp", bufs=2) as xp,
            tc.tile_pool(name="sp", bufs=4) as sp,
            tc.tile_pool(name="fhp", bufs=3) as fhp,
            tc.tile_pool(name="r0p", bufs=2) as r0p,
            tc.tile_pool(name="bp", bufs=4) as bp,
            tc.tile_pool(name="pt", bufs=2, space="PSUM") as ptp,
            tc.tile_pool(name="po", bufs=2, space="PSUM") as pop,
            tc.tile_pool(name="dram", bufs=1, space="DRAM") as dp,
        ):
            # ---- constants ----
            ident = cp.tile([P, P], F32)
            make_identity(nc, ident[:])
            ones1 = cp.tile([1, P], F32)
            nc.gpsimd.memset(ones1[:], 1.0)
            if table_bf16:
                ones1t = cp.tile([1, P], TDT, name="ones1t")
                nc.gpsimd.memset(ones1t[:], 1.0)
            else:
                ones1t = ones1
            zrow = cp.tile([1, D], TDT, name="zrow")
            nc.gpsimd.memset(zrow[:], 0.0)

            def load_w(name, src, kc):
                t = cp.tile([P, kc * D], F32, name=name)
                for kk in range(kc):
                    nc.sync.dma_start(out=t[:, kk * D:(kk + 1) * D],
                                      in_=src[kk * P:(kk + 1) * P, :])
                return t

            wfc_t = load_w("wfc_t", Wfc, KCN)
            wbc_t = load_w("wbc_t", Wbc, KCN)
            we_t = load_w("we_t", Wedge, KCE)

            def load_flat(name, src, shape, dt):
                t = cp.tile(shape, dt, name=name)
                nc.sync.dma_start(out=t[:], in_=src[:])
                return t

            bfc_t = load_flat("bfc_t", bfc, [1, D], F32)
            bbc_t = load_flat("bbc_t", bbc, [1, D], F32)
            be_t = load_flat("be_t", bedge, [1, D], F32)
            idx_dram = {"fw_adj": fw_adj_g, "bw_adj": bw_adj_g,
                        "fw_eid": fw_eid_g, "bw_eid": bw_eid_g,
                        "fw_dep": fw_dep_g, "bw_dep": bw_dep_g}
            idx_sb = None
            if table_bf16:
                idx_sb = {nm: load_flat(f"ti_{nm}", t, [P, t.shape[1]], I16)
                          for nm, t in idx_dram.items()}

            def load_blocked(name, src, nb, w, dt):
                tt = cp.tile([P, nb * w], dt, name=name)
                for b in range(nb):
                    nc.sync.dma_start(out=tt[:, b * w:(b + 1) * w],
                                      in_=src[b * P:(b + 1) * P, :])
                return tt

            rc_t = {nm: load_blocked(f"t_{nm}", t, nb, 1, F32)
                    for nm, t, nb in (("rcn_fw", rcn_fw, NB), ("rcn_bw", rcn_bw, NB),
                                      ("rce_fw", rce_fw, EB), ("rce_bw", rce_bw, EB),
                                      ("ce_fw", ce_fw, NB), ("ce_bw", ce_bw, NB))}

            # ---- DRAM tables (internal) ----
            def mk_tab(name, rows):
                return dp.tile([rows, D], TDT, addr_space="Shared", name=name)

            fw_tabA = mk_tab("fw_tabA", NT)
            fw_tabB = mk_tab("fw_tabB", NT)
            bw_tabA = mk_tab("bw_tabA", NT)
            bw_tabB = mk_tab("bw_tabB", NT)
            e_tabA = mk_tab("e_tabA", E)
            e_tabB = mk_tab("e_tabB", E)
            fw_sh = dp.tile([NS, D], F32, name="fw_sh")
            bw_sh = dp.tile([NS, D], F32, name="bw_sh")
            e_sh = dp.tile([ES, D], F32, name="e_sh")
            # shard AG buffers carry a trailing zero row; the AllGather
            # interleaves them into the table as the per-core pad rows
            fw_shB = dp.tile([NS + 1, D], TDT, name="fw_shB")
            bw_shB = dp.tile([NS + 1, D], TDT, name="bw_shB")
            e_shB = dp.tile([ES, D], TDT, name="e_shB") if table_bf16 else e_sh
            nc.sync.dma_start(out=fw_shB[NS:NS + 1, :], in_=zrow[:])
            nc.sync.dma_start(out=bw_shB[NS:NS + 1, :], in_=zrow[:])

            fw_ntabs = [fw_tab0, fw_tabA, fw_tabB]
            bw_ntabs = [bw_tab0, bw_tabA, bw_tabB]
            e_tabs = [e_tab0, e_tabA, e_tabB]

            def bcast0(tab):
                """[128, D] tile with every partition = tab row 0."""
                r0 = r0p.tile([1, D], TDT, name="r0", tag="r0")
                nc.sync.dma_start(out=r0[:], in_=tab[0:1, :])
                ps = ptp.tile([P, D], F32, name="ps_t", tag="ps_t")
                nc.tensor.matmul(out=ps[:], lhsT=ones1t[:], rhs=r0[:],
                                 start=True, stop=True)
                b = bp.tile([P, D], F32, name="b0", tag="b0")
                nc.vector.tensor_copy(out=b[:], in_=ps[:])
                return b

            qctr = [0]

            def gather_q(tab, idx_nm, coloff, nj):
                if idx_sb is not None:
                    isl = idx_sb[idx_nm]
                    isl_ap = isl[:, coloff:coloff + nj * 8]
                else:
                    t = ip.tile([P, nj * 8], I16, name="isl", tag="isl")
                    nc.sync.dma_start(
                        out=t[:], in_=idx_dram[idx_nm][:, coloff:coloff + nj * 8])
                    isl_ap = t[:]
                g = gp.tile([P, 4 * D], TDT, name="g", tag="g")
                qctr[0] = (qctr[0] + 1) % 4
                nc.gpsimd.dma_gather(
                    out_ap=g[:, 0:nj * D].rearrange("p (t e) -> p t e", e=D),
                    in_ap=tab[:],
                    idxs_ap=isl_ap,
                    num_idxs=nj * P,
                    num_idxs_reg=nj * P,
                    elem_size=D,
                    queue_num=qctr[0],
                )
                return g

            def gather_stream(tab, idx_nm, base, ks):
                """Split a ks-slot stream into <=4-slot gather calls."""
                gts = []
                s0 = 0
                while s0 < ks:
                    nj = min(4, ks - s0)
                    gts.append((gather_q(tab, idx_nm, base + s0 * 8, nj), nj))
                    s0 += nj
                return gts

            ADD = mybir.AluOpType.add

            def mean_sbuf(gtiles, rc, b, corrs):
                """sm[128,D] = rc_b * (sum_j G_j + sum_i corr_i * B0_i)."""
                sm = sp.tile([P, D], F32, name="sm", tag="sm")
                init = False
                for (cx, B0x) in corrs:
                    if not init:
                        nc.vector.tensor_scalar_mul(sm[:], B0x[:], cx[:, b:b + 1])
                        init = True
                    else:
                        ct = sp.tile([P, D], F32, name="ct", tag="ct")
                        nc.vector.tensor_scalar_mul(ct[:], B0x[:], cx[:, b:b + 1])
                        nc.vector.tensor_tensor(out=sm[:], in0=sm[:], in1=ct[:], op=ADD)
                for g, nj in gtiles:
                    if nj == 1:
                        if not init:
                            nc.vector.tensor_copy(out=sm[:], in_=g[:, 0:D])
                            init = True
                        else:
                            tq = sp.tile([P, D], F32, name="tq1", tag="tq1")
                            nc.vector.tensor_copy(out=tq[:], in_=g[:, 0:D])
                            nc.vector.tensor_tensor(
                                out=sm[:], in0=sm[:], in1=tq[:], op=ADD)
                        continue
                    tq = sp.tile([P, 2 * D], F32, name="tq", tag="tq")
                    if nj == 4:
                        nc.vector.tensor_tensor(
                            out=tq[:], in0=g[:, 0:2 * D], in1=g[:, 2 * D:4 * D],
                            op=ADD)
                        nc.vector.tensor_tensor(
                            out=tq[:, 0:D], in0=tq[:, 0:D], in1=tq[:, D:2 * D],
                            op=ADD)
                    elif nj == 3:
                        nc.vector.tensor_tensor(
                            out=tq[:, 0:D], in0=g[:, 0:D], in1=g[:, D:2 * D],
                            op=ADD)
                        nc.vector.tensor_copy(out=tq[:, D:2 * D],
                                              in_=g[:, 2 * D:3 * D])
                        nc.vector.tensor_tensor(
                            out=tq[:, 0:D], in0=tq[:, 0:D], in1=tq[:, D:2 * D],
                            op=ADD)
                    else:  # nj == 2
                        nc.vector.tensor_tensor(
                            out=tq[:, 0:D], in0=g[:, 0:D], in1=g[:, D:2 * D],
                            op=ADD)
                    if not init:
                        nc.vector.tensor_copy(out=sm[:], in_=tq[:, 0:D])
                        init = True
                    else:
                        nc.vector.tensor_tensor(
                            out=sm[:], in0=sm[:], in1=tq[:, 0:D], op=ADD)
                assert init
                nc.vector.tensor_scalar_mul(sm[:], sm[:], rc[:, b:b + 1])
                return sm

            def transpose_into(xT, cbase, src_sb):
                pt = ptp.tile([P, DC * P], F32, name="ps_t")
                for c in range(DC):
                    nc.tensor.transpose(
                        out=pt[:, c * P:(c + 1) * P], in_=src_sb[:, c * P:(c + 1) * P],
                        identity=ident[:],
                    )
                nc.vector.tensor_copy(
                    out=xT[:, cbase * P:(cbase + DC) * P], in_=pt[:],
                )

            def linear(xT, kc, w_t, b_row, relu, out_sb):
                ps = pop.tile([P, D], F32, name="ps_o")
                for kk in range(kc):
                    nc.tensor.matmul(
                        out=ps[:], lhsT=xT[:, kk * P:(kk + 1) * P],
                        rhs=w_t[:, kk * D:(kk + 1) * D],
                        start=(kk == 0), stop=False,
                    )
                nc.tensor.matmul(
                    out=ps[:], lhsT=ones1[:], rhs=b_row[:], start=False, stop=True,
                )
                if relu:
                    nc.vector.tensor_scalar_max(out_sb[:], ps[:], 0.0)
                else:
                    nc.vector.tensor_copy(out=out_sb[:], in_=ps[:])

            def node_block(k, b, ntab, etab, own_src, a_nm, e_nm, rc_nm,
                           ce_nm, aoffs, akept, eoffs, ekept,
                           w_t, b_row, dst, dstB, eB0):
                relu = (k < K - 1)
                gts = gather_stream(ntab, a_nm, aoffs[b], akept[b])
                gts += gather_stream(etab, e_nm, eoffs[b], ekept[b])
                nf = sp.tile([P, D], F32, name="nf", tag="nf")
                nc.sync.dma_start(out=nf[:], in_=own_src[b * P:(b + 1) * P, :])

                sm = mean_sbuf(gts, rc_t[rc_nm], b, [(rc_t[ce_nm], eB0)])

                xT = xp.tile([P, KCN * P], F32, name="xT", tag="xT")
                transpose_into(xT, 0, nf)
                transpose_into(xT, DC, sm)

                fh = fhp.tile([P, D], F32, name="fh", tag="fh")
                linear(xT, KCN, w_t, b_row, relu, fh)
                nc.sync.dma_start(out=dst[b * P:(b + 1) * P, :], in_=fh[:])
                if dstB is not None:
                    fhb = fhp.tile([P, D], TDT, name="fhb", tag="fhb")
                    nc.vector.tensor_copy(out=fhb[:], in_=fh[:])
                    nc.sync.dma_start(out=dstB[b * P:(b + 1) * P, :], in_=fhb[:])

            def edge_block(u, b, fw_nt, bw_nt, own_src):
                gf = gather_stream(fw_nt, "fw_dep", of_e[b], kf_e[b])
                gb = gather_stream(bw_nt, "bw_dep", ob_e[b], kb_e[b])
                eo = sp.tile([P, D], F32, name="eo", tag="nf")
                nc.sync.dma_start(out=eo[:], in_=own_src[b * P:(b + 1) * P, :])

                smf = mean_sbuf(gf, rc_t["rce_fw"], b, [])
                smb = mean_sbuf(gb, rc_t["rce_bw"], b, [])

                xT = xp.tile([P, KCE * P], F32, name="xTe", tag="xT")
                transpose_into(xT, 0, eo)
                transpose_into(xT, DC, smf)
                transpose_into(xT, 2 * DC, smb)

                es = fhp.tile([P, D], F32, name="es", tag="fh")
                linear(xT, KCE, we_t, be_t, True, es)
                nc.sync.dma_start(out=e_sh[b * P:(b + 1) * P, :], in_=es[:])
                if table_bf16:
                    esb = fhp.tile([P, D], TDT, name="esb", tag="fhb")
                    nc.vector.tensor_copy(out=esb[:], in_=es[:])
                    nc.sync.dma_start(out=e_shB[b * P:(b + 1) * P, :], in_=esb[:])


            def allgather(src, dst_ap):
                nc.gpsimd.collective_compute(
                    "AllGather", mybir.AluOpType.bypass, replica_groups=rg,
                    ins=[src], outs=[dst_ap],
                )

            def allgather_e(dsttab):
                allgather(e_shB[:], dsttab[:])

            for k in range(K):
                last = (k == K - 1)
                fw_dst = fw_sh if not last else fw_out
                bw_dst = bw_sh if not last else bw_out
                fw_ownsrc = fw_own0 if k == 0 else fw_sh
                bw_ownsrc = bw_own0 if k == 0 else bw_sh
                eB0 = bcast0(e_tabs[k])
                for b in range(NB):
                    node_block(k, b, fw_ntabs[k], e_tabs[k], fw_ownsrc,
                               "fw_adj", "fw_eid", "rcn_fw", "ce_fw",
                               oa_fw, ka_fw, oe_fw, ke_fw,
                               wfc_t, bfc_t, fw_dst,
                               fw_shB if not last else None, eB0)
                if not last:
                    allgather(fw_shB[:], fw_ntabs[k + 1][:])
                for b in range(NB):
                    node_block(k, b, bw_ntabs[k], e_tabs[k], bw_ownsrc,
                               "bw_adj", "bw_eid", "rcn_bw", "ce_bw",
                               oa_bw, ka_bw, oe_bw, ke_bw,
                               wbc_t, bbc_t, bw_dst,
                               bw_shB if not last else None, eB0)
                if not last:
                    allgather(bw_shB[:], bw_ntabs[k + 1][:])
                    e_ownsrc = e_own0 if k == 0 else e_sh
                    for b in range(EB):
                        edge_block(k, b, fw_ntabs[k + 1], bw_ntabs[k + 1],
                                   e_ownsrc)
                    allgather_e(e_tabs[k + 1])

    nc.compile()
    return nc


def _valid_first(idx):
    """Per-row: valid entries first (stable), -1s last."""
    key = (idx < 0).astype(np.int32)
    order = np.argsort(key, axis=1, kind="stable")
    return np.take_along_axis(idx, order, axis=1)


def _pack_stream(rows, kept, pad_val):
    """rows [R, W] valid-first idx; kept[b] 4-slot chunks per 128-block.
    Returns [128, sum(kept)*32] int16 wrapped layout (pad -> pad_val)."""
    nb = rows.shape[0] // P
    cols = []
    for b in range(nb):
        k4 = kept[b]
        sub = rows[b * P:(b + 1) * P, :k4].copy()
        sub[sub < 0] = pad_val
        lst = sub.T.reshape(-1)                 # slot-major: i = j*128 + p
        wrapped = lst.reshape(-1, 16).T         # [16, k4*8]
        cols.append(np.tile(wrapped, (8, 1)).astype(np.int16))
    return np.concatenate(cols, axis=1) if cols else \
        np.zeros((P, 0), np.int16)


def _kept_chunks(deg, nb, min_chunks=0):
    """Per 128-block: exact max-degree SLOT count."""
    out = []
    for b in range(nb):
        w = int(deg[b * P:(b + 1) * P].max()) if deg.size else 0
        out.append(max(min_chunks, w))
    return out


def prep_inputs(cfg: Cfg, inputs: dict, table_bf16=True):
    N, E, D, DEG, DEP, CORES = cfg.N, cfg.E, cfg.D, cfg.DEG, cfg.DEP, cfg.CORES
    NS, ES, NB, EB = cfg.NS, cfg.ES, cfg.NB, cfg.EB
    f32 = np.float32

    fw = np.ascontiguousarray(np.asarray(inputs["fw_input"], f32))
    bw = np.ascontiguousarray(np.asarray(inputs["bw_input"], f32))
    ee = np.ascontiguousarray(np.asarray(inputs["edge_embs"], f32))

    idxs = {k: np.asarray(inputs[k], np.int64) for k in
            ("fw_adj", "bw_adj", "fw_edgeid", "bw_edgeid",
             "fw_edgedep", "bw_edgedep")}
    deg = {k: (v >= 0).sum(1) for k, v in idxs.items()}

    # ---- global degree sort, blocks dealt round-robin to cores so every
    # core's block b sees the same degree profile (kept counts stay tight) --
    def dealt_order(primary, secondary, rows_per):
        p1 = primary
        key = (-(p1 > (p1.max() // 2)).astype(np.int64)) * 10**6 - secondary
        order = np.argsort(key, kind="stable")           # new global -> old
        blocks = order.reshape(-1, P)
        return np.concatenate(
            [blocks[c::CORES].reshape(-1) for c in range(CORES)])

    ord_fw = dealt_order(deg["fw_adj"], deg["fw_edgeid"], NS)
    ord_bw = dealt_order(deg["bw_adj"], deg["bw_edgeid"], NS)
    ord_e = dealt_order(deg["fw_edgedep"], deg["bw_edgedep"], ES)

    def gmap(order, rows_per, stride=None):
        # old id order[c*rows_per+i] -> new TABLE id c*stride + i
        st = rows_per if stride is None else stride
        n = order.size
        cores = n // rows_per
        tgt = np.concatenate(
            [c * st + np.arange(rows_per) for c in range(cores)])
        m = np.empty(n, np.int64)
        m[order] = tgt
        return m

    gm_fw = gmap(ord_fw, NS, NS + 1)
    gm_bw = gmap(ord_bw, NS, NS + 1)
    gm_e = gmap(ord_e, ES)

    perm_fw, perm_bw = ord_fw, ord_bw

    def permute_rows(a, order, rows_per=None):
        return a[order]

    # remap index values (keep -1), then permute rows
    def remap(v, m):
        return np.where(v >= 0, m[np.maximum(v, 0)], -1)

    adj_fw = permute_rows(remap(idxs["fw_adj"], gm_fw), ord_fw)
    adj_bw = permute_rows(remap(idxs["bw_adj"], gm_bw), ord_bw)
    eid_fw = permute_rows(remap(idxs["fw_edgeid"], gm_e), ord_fw)
    eid_bw = permute_rows(remap(idxs["bw_edgeid"], gm_e), ord_bw)
    dep_fw = permute_rows(remap(idxs["fw_edgedep"], gm_fw), ord_e)
    dep_bw = permute_rows(remap(idxs["bw_edgedep"], gm_bw), ord_e)

    fw_s = permute_rows(fw, ord_fw)
    bw_s = permute_rows(bw, ord_bw)
    ee_s = permute_rows(ee, ord_e)

    deg_adj_fw = permute_rows(deg["fw_adj"], ord_fw)
    deg_eid_fw = permute_rows(deg["fw_edgeid"], ord_fw)
    deg_adj_bw = permute_rows(deg["bw_adj"], ord_bw)
    deg_eid_bw = permute_rows(deg["bw_edgeid"], ord_bw)
    deg_f_e = permute_rows(deg["fw_edgedep"], ord_e)
    deg_b_e = permute_rows(deg["bw_edgedep"], ord_e)

    adj_fw = _valid_first(adj_fw)
    adj_bw = _valid_first(adj_bw)
    eid_fw = _valid_first(eid_fw)
    eid_bw = _valid_first(eid_bw)
    dep_fw = _valid_first(dep_fw)
    dep_bw = _valid_first(dep_bw)

    if table_bf16:
        import ml_dtypes
        tdt = ml_dtypes.bfloat16
    else:
        tdt = f32
    zrow = np.zeros((1, D), f32)

    def mk_ntab(a):
        # interleave a zero row after each core's shard
        parts = []
        for c in range(CORES):
            parts.append(a[c * NS:(c + 1) * NS])
            parts.append(zrow)
        return np.concatenate(parts, 0).astype(tdt)

    fw_tab = mk_ntab(fw_s)
    bw_tab = mk_ntab(bw_s)
    ee_tab = ee_s.astype(tdt)

    def rec(c):
        with np.errstate(divide="ignore"):
            return (1.0 / c.astype(f32)).astype(f32)

    rcn_fw_f = rec(deg_adj_fw + deg_eid_fw)
    rcn_bw_f = rec(deg_adj_bw + deg_eid_bw)
    rce_fw_f = rec(deg_f_e)
    rce_bw_f = rec(deg_b_e)

    meta = {}
    per_core = {c: {} for c in range(CORES)}
    kept_all = {}
    for nm, rows, degv, rows_per, nb in (
            ("kept_adj_fw", adj_fw, deg_adj_fw, NS, NB),
            ("kept_eid_fw", eid_fw, deg_eid_fw, NS, NB),
            ("kept_adj_bw", adj_bw, deg_adj_bw, NS, NB),
            ("kept_eid_bw", eid_bw, deg_eid_bw, NS, NB),
            ("kept_f", dep_fw, deg_f_e, ES, EB),
            ("kept_b", dep_bw, deg_b_e, ES, EB)):
        # kept chunks must be IDENTICAL across cores (single SPMD program):
        # take per-core max per block index
        kept_pc = [_kept_chunks(degv[c * rows_per:(c + 1) * rows_per], nb)
                   for c in range(CORES)]
        kept = [max(kept_pc[c][b] for c in range(CORES)) for b in range(nb)]
        meta[nm] = kept
        kept_all[nm] = kept

    # eid row-0 correction: counts row-0 pads inside kept chunks
    def ce_vals(deg_e, kept, rows_per):
        out = np.empty(rows_per * CORES, f32)
        for c in range(CORES):
            for b in range(len(kept)):
                sl = slice(c * rows_per + b * P, c * rows_per + (b + 1) * P)
                out[sl] = -(kept[b] - deg_e[sl])
        return out

    ce_fw_f = ce_vals(deg_eid_fw, kept_all["kept_eid_fw"], NS)
    ce_bw_f = ce_vals(deg_eid_bw, kept_all["kept_eid_bw"], NS)

    Wfc = np.ascontiguousarray(np.asarray(inputs["Wfc"], f32))
    Wbc = np.ascontiguousarray(np.asarray(inputs["Wbc"], f32))
    Wedge = np.ascontiguousarray(np.asarray(inputs["Wedge"], f32))
    bfc = np.asarray(inputs["bfc"], f32).reshape(1, D)
    bbc = np.asarray(inputs["bbc"], f32).reshape(1, D)
    bedge = np.asarray(inputs["bedge"], f32).reshape(1, D)

    in_maps = []
    for c in range(CORES):
        nsl = slice(c * NS, (c + 1) * NS)
        esl = slice(c * ES, (c + 1) * ES)
        in_maps.append({
            "fw_tab0": fw_tab, "bw_tab0": bw_tab, "e_tab0": ee_tab,
            "fw_own0": fw_s[nsl].copy(), "bw_own0": bw_s[nsl].copy(),
            "e_own0": ee_s[esl].copy(),
            "fw_adj_g": _pack_stream(adj_fw[nsl], meta["kept_adj_fw"], NS),
            "bw_adj_g": _pack_stream(adj_bw[nsl], meta["kept_adj_bw"], NS),
            "fw_eid_g": _pack_stream(eid_fw[nsl], meta["kept_eid_fw"], 0),
            "bw_eid_g": _pack_stream(eid_bw[nsl], meta["kept_eid_bw"], 0),
            "fw_dep_g": _pack_stream(dep_fw[esl], meta["kept_f"], NS),
            "bw_dep_g": _pack_stream(dep_bw[esl], meta["kept_b"], NS),
            "rcn_fw": rcn_fw_f[nsl, None].copy(), "rcn_bw": rcn_bw_f[nsl, None].copy(),
            "rce_fw": rce_fw_f[esl, None].copy(), "rce_bw": rce_bw_f[esl, None].copy(),
            "ce_fw": ce_fw_f[nsl, None].copy(), "ce_bw": ce_bw_f[nsl, None].copy(),
            "Wfc": Wfc, "Wbc": Wbc, "Wedge": Wedge,
            "bfc": bfc, "bbc": bbc, "bedge": bedge,
        })
    return in_maps, meta, (perm_fw, perm_bw)


def assemble_outputs(cfg: Cfg, results, perms):
    ord_fw, ord_bw = perms
    fw = np.concatenate([results[c]["fw_out"] for c in range(cfg.CORES)], axis=0)
    bw = np.concatenate([results[c]["bw_out"] for c in range(cfg.CORES)], axis=0)
    ofw = np.empty_like(fw)
    obw = np.empty_like(bw)
    ofw[ord_fw] = fw
    obw[ord_bw] = bw
    return ofw, obw


# ======================= self-contained runner =======================
import os as _os
import types as _types


def _install_axon_prof():
    """Provide antenv.axon_hooks + NTFF hook so trace=True works under axon."""
    name = "antenv.axon_hooks"
    if name in sys.modules:
        return True
    try:
        mod = _types.ModuleType(name)
        mod._hook = None
        mod.set_axon_ntff_profile_hook = lambda h: setattr(mod, "_hook", h)
        mod.get_axon_ntff_profile_hook = lambda: mod._hook
        sys.modules[name] = mod
        import antenv
        antenv.axon_hooks = mod
        from trn_agent_boot.trn_boot import _ntff_profile_via_ctypes
        mod.set_axon_ntff_profile_hook(
            _ntff_profile_via_ctypes('/opt/axon/libaxon_pjrt.so'))
        return True
    except Exception:
        sys.modules.pop(name, None)
        return False


_CACHE = {}
LAST_EXEC_NS = None
LAST_PROFILE = None


def kernel(**inputs):
    """Full-input GNN forward on 8 TRN2 NeuronCores. Returns (fw, bw)."""
    global LAST_EXEC_NS, LAST_PROFILE
    from concourse.bass_utils import run_bass_kernel_spmd

    cfg = Cfg()
    bf16 = _os.environ.get("GNN_F32", "0") != "1"
    in_maps, meta, perms = prep_inputs(cfg, inputs, table_bf16=bf16)
    key = ("nc", tuple(tuple(meta[k]) for k in sorted(meta)))
    if _CACHE.get("key") != key:
        _CACHE["nc"] = build(cfg, meta, table_bf16=bf16)
        _CACHE["key"] = key
    nc = _CACHE["nc"]

    profile = _os.environ.get("GNN_PROFILE", "0") == "1"
    if profile:
        profile = _install_axon_prof()
    res = run_bass_kernel_spmd(nc, in_maps, core_ids=list(range(cfg.CORES)),
                               trace=profile)
    LAST_EXEC_NS = res.exec_time_ns
    LAST_PROFILE = res.profile_json
    if res.instructions_and_trace is not None:
        try:
            print("trace:", res.instructions_and_trace[1])
        except Exception:
            pass
    return assemble_outputs(cfg, res.results, perms)
